# revision 22
# baseline (speedup 1.0000x reference)
"""Multi-head self-attention (B=2, S=2048, D=1024, H=16, causal) on 8 NeuronCores.

Sharding: 32 (batch, head) instances -> 4 heads of one batch per core
(cores 0-3: batch 0, cores 4-7: batch 1; core c owns heads 4*(c%4) .. +3).
Wq/Wk/Wv are split by rows (head dims), Wo by columns; each core computes a
partial y[b] = attn_out_heads @ Wo_cols.T and the host sums the 4 partials
per batch at gather time (tensor-parallel reduce).

Per-core kernel. All matmuls fp16 x fp16 -> fp32 psum (fp16 stationary
operands get fast, reorder-hidden LDWEIGHTS; float32r would force a fused
half-rate weight load serialized with every matmul). No on-device transposes:
  QT[256,2048] = wqT.T @ xT        (head-pair tiles: rows 0-63 / 64-127)
  KT likewise; V[2048,256] natural (lhsT = xT chunks), augmented with a
  ones column per head -> va tiles [128, 4*65].
  Scores computed transposed, blockwise [k-tile 128, q-chunk 512]:
      S^T = KT_h.T @ QT_h   -- two heads row-packed (contraction d=64 at
      partition bases 0 / 64). Both heads of a k-tile share one [128,1024]
      psum tile (bufs=2 -> exp overlaps the next k-tile's score matmuls);
      one Exp (scale=1/8) per tile on ScalarE -> P fp16 in SBUF.
  Causal masking only on diagonal k-tiles via precomputed 0/1 mask multiply.
  AV: lhsT = [V_h | 1] fp16 [k,65], rhs = P [k,512] -> psum [65,512]
      accumulated over k-tiles = unnormalized out^T (rows 0-63) + softmax
      denominators (row 64). Normalize columns via reciprocal_approx_fast +
      matmul partition broadcast + DVE multiply -> out_headsT [256,2048] fp16.
  y = out_headsT.T @ woT -> [2048, 1024] fp16 partial, DMA'd out
      (host accumulates partials in fp32).

Pipeline over q-chunks. The next chunk's QKV projections are emitted after
chunk n's two attention k-loops but before hp1's normalize + output
projection, so the tensor engine stays busy (and the PE clock-gate stays
warm) through the normalize tail. Projections are never interleaved with
attention's score/AV psum accumulation groups (nondeterministic hardware
corruption when they are; verified on HW).
"""
import os
import sys

sys.path.insert(0, "/opt/trn_rl_repo")

import numpy as np

import concourse.bass as bass  # noqa: F401
import concourse.mybir as mybir
from concourse import bacc
from concourse.tile import TileContext
from concourse.bass_utils import run_bass_kernel_spmd

B, S, D = 2, 2048, 1024
H, HD = 16, 64
NCORES = 8
HPC = 4            # heads per core
SC = 512           # q-chunk width
NQC = S // SC      # 4 q-chunks
NKT = S // 128     # 16 k-tiles
F16 = mybir.dt.float16
F32 = mybir.dt.float32
ATTN_SCALE = 1.0 / np.sqrt(HD)

_CACHE = {}


def _build():
    nc = bacc.Bacc("TRN2", target_bir_lowering=False, debug=False, num_devices=NCORES)

    xT_d = nc.declare_dram_parameter("xT", [D, S], F16, isOutput=False)
    wqT_d = nc.declare_dram_parameter("wqT", [D, 256], F16, isOutput=False)
    wkT_d = nc.declare_dram_parameter("wkT", [D, 256], F16, isOutput=False)
    wvT_d = nc.declare_dram_parameter("wvT", [D, 256], F16, isOutput=False)
    woT_d = nc.declare_dram_parameter("woT", [256, D], F16, isOutput=False)
    mask_d = nc.declare_dram_parameter("mask", [4, 128, SC], F16, isOutput=False)
    ones_d = nc.declare_dram_parameter("ones", [128, HPC], F16, isOutput=False)
    y_d = nc.declare_dram_parameter("y", [S, D], F16, isOutput=True)

    with TileContext(nc) as tc:
        with (
            tc.tile_pool(name="static", bufs=1) as st,
            tc.tile_pool(name="ppool", bufs=10) as ppool,
            tc.tile_pool(name="rbpool", bufs=6) as rbpool,
            tc.tile_pool(name="recpool", bufs=4) as recpool,
            tc.tile_pool(name="ystage", bufs=4) as ystage,
            tc.tile_pool(name="psA", bufs=2, space="PSUM") as psA,
            tc.tile_pool(name="psS", bufs=2, space="PSUM") as psS,
            tc.tile_pool(name="psV", bufs=1, space="PSUM") as psV,
        ):
            # ---- input DMAs, ordered for earliest proj(0) start:
            # sync queue:   xT chunk 0, wk, mask, xT chunks 1-3
            # gpsimd queue: wq, wv, wo
            mask = st.tile([128, 4 * SC], F16, name="mask", tag="mask")
            wq = st.tile([128, 2048], F16, name="wq", tag="wq")
            wk = st.tile([128, 2048], F16, name="wk", tag="wk")
            wv = st.tile([128, 2048], F16, name="wv", tag="wv")
            wo = st.tile([128, 2048], F16, name="wo", tag="wo")
            xT = [st.tile([128, S], F16, name=f"xT{k}", tag=f"xT{k}") for k in range(8)]
            for k in range(8):
                nc.sync.dma_start(
                    out=xT[k][:, 0:SC], in_=xT_d[128 * k : 128 * k + 128, 0:SC]
                )
            for wt, wd, eng in (
                (wq, wqT_d, nc.gpsimd),
                (wk, wkT_d, nc.sync),
                (wv, wvT_d, nc.gpsimd),
            ):
                for k in range(8):
                    eng.dma_start(
                        out=wt[:, 256 * k : 256 * k + 256],
                        in_=wd[128 * k : 128 * k + 128, :],
                    )
            for t in range(4):
                nc.sync.dma_start(out=mask[:, SC * t : SC * t + SC], in_=mask_d[t])
            for cc in range(2):
                nc.gpsimd.dma_start(
                    out=wo[:, 1024 * cc : 1024 * cc + 1024],
                    in_=woT_d[128 * cc : 128 * cc + 128, :],
                )
            for n in range(1, NQC):
                for k in range(8):
                    nc.sync.dma_start(
                        out=xT[k][:, SC * n : SC * n + SC],
                        in_=xT_d[128 * k : 128 * k + 128, SC * n : SC * n + SC],
                    )

            QT = [st.tile([128, S], F16, name=f"QT{m}", tag=f"QT{m}") for m in range(2)]
            KT = [st.tile([128, S], F16, name=f"KT{m}", tag=f"KT{m}") for m in range(2)]
            va = [
                st.tile([128, 65 * HPC], F16, name=f"va{i}", tag=f"va{i}")
                for i in range(NKT)
            ]
            outT = [
                st.tile([128, S], F16, name=f"outT{m}", tag=f"outT{m}")
                for m in range(2)
            ]

            def proj_qk_chunk(n):
                for dst, w in ((QT, wq), (KT, wk)):
                    for m in range(2):
                        acc = psA.tile([128, SC], F32, name="acc", tag="acc")
                        for k in range(8):
                            nc.tensor.matmul(
                                acc[:],
                                w[:, 256 * k + 128 * m : 256 * k + 128 * m + 128],
                                xT[k][:, SC * n : SC * n + SC],
                                start=(k == 0),
                                stop=(k == 7),
                            )
                        nc.vector.tensor_copy(dst[m][:, SC * n : SC * n + SC], acc[:])

            def proj_v(i):
                accv = psA.tile([128, 256], F32, name="accv", tag="acc")
                for k in range(8):
                    nc.tensor.matmul(
                        accv[:],
                        xT[k][:, 128 * i : 128 * i + 128],
                        wv[:, 256 * k : 256 * k + 256],
                        start=(k == 0),
                        stop=(k == 7),
                    )
                for h in range(HPC):
                    nc.vector.tensor_copy(
                        va[i][:, 65 * h : 65 * h + 64], accv[:, 64 * h : 64 * h + 64]
                    )
                ones_ap = va[i].rearrange("p (h c) -> p h c", c=65)[:, :, 64]
                nc.sync.dma_start(out=ones_ap, in_=ones_d[:])

            def attn_kloop(jq, hp):
                """Scores+exp+AV for one head pair; av psum tiles returned
                unnormalized (rows 0-63 = out^T, row 64 = denominators)."""
                nkt = 4 * jq + 4  # causal: k-tiles 0 .. 4*jq+3
                av = [
                    psV.tile([65, SC], F32, name=f"av{u}", tag=f"av{u}")
                    for u in range(2)
                ]
                for kt in range(nkt):
                    sp = psS.tile([128, 1024], F32, name="sp", tag="sp")
                    for u, base in ((0, 0), (1, 64)):
                        nc.tensor.matmul(
                            sp[:, 512 * u : 512 * u + 512],
                            KT[hp][base : base + 64, 128 * kt : 128 * kt + 128],
                            QT[hp][base : base + 64, SC * jq : SC * jq + SC],
                            start=True,
                            stop=True,
                        )
                    pt = ppool.tile([128, 1024], F16, name="pt", tag="pt")
                    nc.scalar.activation(
                        pt[:],
                        sp[:],
                        mybir.ActivationFunctionType.Exp,
                        scale=float(ATTN_SCALE),
                    )
                    t = kt - 4 * jq
                    if t >= 0:  # diagonal k-tile: causal mask
                        for u in range(2):
                            sl = slice(512 * u, 512 * u + 512)
                            nc.vector.tensor_mul(
                                pt[:, sl], pt[:, sl], mask[:, SC * t : SC * t + SC]
                            )
                    for u in range(2):
                        h = 2 * hp + u
                        nc.tensor.matmul(
                            av[u][:],
                            va[kt][:, 65 * h : 65 * h + 65],
                            pt[:, 512 * u : 512 * u + 512],
                            start=(kt == 0),
                            stop=(kt == nkt - 1),
                        )
                return av

            # normalize columns by softmax denominators (row 64), split in
            # two stages so other work can be emitted between them:
            # stage 1 (DVE only): evacuate av to SBUF (this read is what
            # frees the av psum slots for the next k-loop) and compute the
            # fp16 reciprocal row. stage 2 (1 matmul + 1 DVE mul per head):
            # broadcast 1/den across 64 partitions via a K=1 matmul against
            # a ones row (mask pattern 0 row 0 is all-ones), multiply.
            def attn_norm_s1(av):
                st = []
                for u in range(2):
                    avs = rbpool.tile([65, SC], F32, name="avs", tag="avs")
                    nc.vector.tensor_copy(avs[:], av[u][:])
                    den = recpool.tile([1, SC], F32, name="den", tag="den")
                    nc.vector.tensor_copy(den[:], avs[64:65, :])
                    rec = recpool.tile([1, SC], F32, name="rec", tag="rec")
                    nc.vector.reciprocal_approx_fast(rec[:], den[:])
                    rec16 = recpool.tile([1, SC], F16, name="rec16", tag="rec16")
                    nc.vector.tensor_copy(rec16[:], rec[:])
                    st.append((avs, rec16))
                return st

            def attn_norm_s2(jq, hp, st):
                for u, (avs, rec16) in enumerate(st):
                    rbp = psA.tile([64, SC], F32, name="rbp", tag="acc")
                    nc.tensor.matmul(
                        rbp[:], mask[0:1, 0:64], rec16[:], start=True, stop=True
                    )
                    nc.vector.tensor_mul(
                        outT[hp][64 * u : 64 * u + 64, SC * jq : SC * jq + SC],
                        avs[0:64, :],
                        rbp[:],
                    )

            def attn_normalize(jq, hp, av):
                attn_norm_s2(jq, hp, attn_norm_s1(av))

            def wo_chunk(jq):
                for i in range(4 * jq, 4 * jq + 4):
                    for n in range(2):
                        yp = psA.tile([128, 512], F32, name="yp", tag="acc")
                        for cc in range(2):
                            nc.tensor.matmul(
                                yp[:],
                                outT[cc][:, 128 * i : 128 * i + 128],
                                wo[:, 1024 * cc + 512 * n : 1024 * cc + 512 * n + 512],
                                start=(cc == 0),
                                stop=(cc == 1),
                            )
                        ys = ystage.tile([128, 512], F16, name="ys", tag="ys")
                        nc.vector.tensor_copy(ys[:], yp[:])
                        eng = nc.sync if (i % 2 == 0) else nc.gpsimd
                        eng.dma_start(
                            out=y_d[128 * i : 128 * i + 128, 512 * n : 512 * n + 512],
                            in_=ys[:],
                        )

            # NOTE: projections must not interleave with attention's score/AV
            # psum accumulation groups (nondeterministic hardware corruption,
            # verified repeatedly on HW). Emitting proj(n+1) after chunk n's
            # k-loops but before its hp1 normalize + wo keeps the tensor
            # queue fed through the normalize tail without touching the
            # attention groups.
            # Software pipeline over chunks. Per chunk boundary the emission
            # is: [kloop(n,1)] [proj(n+1)+V] [norm(n,0) stage2]
            # [norm(n,1) stage1] [kloop(n+1,0)] [norm(n,1) stage2] [wo(n)].
            # The next chunk's first k-loop sits BEFORE chunk n's remaining
            # bcast/wo work, so the scalar engine's exp stream resumes as
            # soon as the projections land instead of also waiting out the
            # normalize broadcasts and the output projection. All psum
            # accumulation groups are closed at every insertion point.
            mode = os.environ.get("KV_PIPE", "2")
            if mode == "2":
                proj_qk_chunk(0)
                for i in range(4):
                    proj_v(i)
                av0 = attn_kloop(0, 0)
                for n in range(NQC):
                    s1_0 = attn_norm_s1(av0)
                    av1 = attn_kloop(n, 1)
                    if n + 1 < NQC:
                        proj_qk_chunk(n + 1)
                        for i in range(4 * n + 4, 4 * n + 8):
                            proj_v(i)
                    attn_norm_s2(n, 0, s1_0)
                    s1_1 = attn_norm_s1(av1)
                    if n + 1 < NQC:
                        av0 = attn_kloop(n + 1, 0)
                    attn_norm_s2(n, 1, s1_1)
                    wo_chunk(n)
            elif mode == "1":
                proj_qk_chunk(0)
                for i in range(4):
                    proj_v(i)
                for n in range(NQC):
                    av0 = attn_kloop(n, 0)
                    attn_normalize(n, 0, av0)
                    av1 = attn_kloop(n, 1)
                    if n + 1 < NQC:
                        proj_qk_chunk(n + 1)
                        for i in range(4 * n + 4, 4 * n + 8):
                            proj_v(i)
                    attn_normalize(n, 1, av1)
                    wo_chunk(n)
            else:
                for n in range(NQC):
                    proj_qk_chunk(n)
                    for i in range(4 * n, 4 * n + 4):
                        proj_v(i)
                    av0 = attn_kloop(n, 0)
                    attn_normalize(n, 0, av0)
                    av1 = attn_kloop(n, 1)
                    attn_normalize(n, 1, av1)
                    wo_chunk(n)

    nc.compile()
    return nc


def _masks_np():
    m = np.zeros((4, 128, SC), dtype=np.float16)
    qq = np.arange(SC)[None, :]
    kk = np.arange(128)[:, None]
    for t in range(4):
        m[t] = ((128 * t + kk) <= qq).astype(np.float16)
    return m


def kernel(x, Wq, Wk, Wv, Wo):
    x = np.asarray(x, dtype=np.float32)
    Wq = np.asarray(Wq, dtype=np.float32)
    Wk = np.asarray(Wk, dtype=np.float32)
    Wv = np.asarray(Wv, dtype=np.float32)
    Wo = np.asarray(Wo, dtype=np.float32)

    if "nc" not in _CACHE:
        _CACHE["nc"] = _build()
    nc = _CACHE["nc"]

    masks = _masks_np()
    xT = [np.ascontiguousarray(x[b].T).astype(np.float16) for b in range(B)]
    in_maps = []
    for c in range(NCORES):
        b, g = c // 4, c % 4
        rows = slice(256 * g, 256 * g + 256)
        in_maps.append(
            {
                "xT": xT[b],
                "wqT": np.ascontiguousarray(Wq[rows].T).astype(np.float16),
                "wkT": np.ascontiguousarray(Wk[rows].T).astype(np.float16),
                "wvT": np.ascontiguousarray(Wv[rows].T).astype(np.float16),
                "woT": np.ascontiguousarray(Wo[:, rows].T).astype(np.float16),
                "mask": masks,
                "ones": np.ones((128, HPC), dtype=np.float16),
            }
        )

    trace = False
    if os.environ.get("KERNEL_TRACE") == "1":
        try:
            from trn_agent_boot.trn_boot import _ntff_profile_via_ctypes

            try:
                from antenv.axon_hooks import (
                    get_axon_ntff_profile_hook,
                    set_axon_ntff_profile_hook,
                )
            except ImportError:
                # this image's antenv lacks axon_hooks; provide the
                # 2-function registry bass_utils expects (test-only path)
                import types

                import antenv

                mod = types.ModuleType("antenv.axon_hooks")
                mod._hook = None

                def set_axon_ntff_profile_hook(h, _m=mod):
                    _m._hook = h

                def get_axon_ntff_profile_hook(_m=mod):
                    return _m._hook

                mod.set_axon_ntff_profile_hook = set_axon_ntff_profile_hook
                mod.get_axon_ntff_profile_hook = get_axon_ntff_profile_hook
                sys.modules["antenv.axon_hooks"] = mod
                antenv.axon_hooks = mod

            if get_axon_ntff_profile_hook() is None:
                set_axon_ntff_profile_hook(
                    _ntff_profile_via_ctypes("/opt/axon/libaxon_pjrt.so")
                )
            trace = True
        except Exception:
            trace = False

    res = run_bass_kernel_spmd(nc, in_maps, core_ids=list(range(NCORES)), trace=trace)
    _CACHE["exec_time_ns"] = res.exec_time_ns
    _CACHE["res"] = res
    y = np.zeros((B, S, D), dtype=np.float32)
    for c in range(NCORES):
        y[c // 4] += res.results[c]["y"].astype(np.float32)
    return y


# revision 23
# speedup vs baseline: 1.2099x; 1.2099x over previous
"""Multi-head self-attention (B=2, S=2048, D=1024, H=16, causal) on 8 NeuronCores.

Sharding: 32 (batch, head) instances -> 4 heads of one batch per core
(cores 0-3: batch 0, cores 4-7: batch 1; core c owns heads 4*(c%4) .. +3).
Wq/Wk/Wv are split by rows (head dims), Wo by columns; each core computes a
partial y[b] = attn_out_heads @ Wo_cols.T and the host sums the 4 partials
per batch at gather time (tensor-parallel reduce).

Per-core kernel. All matmuls fp16 x fp16 -> fp32 psum (fp16 stationary
operands get fast, reorder-hidden LDWEIGHTS; float32r would force a fused
half-rate weight load serialized with every matmul). No on-device transposes:
  QT[256,2048] = wqT.T @ xT        (head-pair tiles: rows 0-63 / 64-127)
  KT likewise; V[2048,256] natural (lhsT = xT chunks), augmented with a
  ones column per head -> va tiles [128, 4*65].
  Scores computed transposed, blockwise [k-tile 128, q-chunk 512]:
      S^T = KT_h.T @ QT_h   -- two heads row-packed (contraction d=64 at
      partition bases 0 / 64). Both heads of a k-tile share one [128,1024]
      psum tile (bufs=2 -> exp overlaps the next k-tile's score matmuls);
      one Exp (scale=1/8) per tile on ScalarE -> P fp16 in SBUF.
  Causal masking only on diagonal k-tiles via precomputed 0/1 mask multiply.
  AV: lhsT = [V_h | 1] fp16 [k,65], rhs = P [k,512] -> psum [65,512]
      accumulated over k-tiles = unnormalized out^T (rows 0-63) + softmax
      denominators (row 64). Normalize columns via reciprocal_approx_fast +
      matmul partition broadcast + DVE multiply -> out_headsT [256,2048] fp16.
  y = out_headsT.T @ woT -> [2048, 1024] fp16 partial, DMA'd out
      (host accumulates partials in fp32).

Pipeline over q-chunks. The next chunk's QKV projections are emitted after
chunk n's two attention k-loops but before hp1's normalize + output
projection, so the tensor engine stays busy (and the PE clock-gate stays
warm) through the normalize tail. Projections are never interleaved with
attention's score/AV psum accumulation groups (nondeterministic hardware
corruption when they are; verified on HW).
"""
import os
import sys

sys.path.insert(0, "/opt/trn_rl_repo")

import numpy as np

import concourse.bass as bass  # noqa: F401
import concourse.mybir as mybir
from concourse import bacc
from concourse.tile import TileContext
from concourse.bass_utils import run_bass_kernel_spmd

B, S, D = 2, 2048, 1024
H, HD = 16, 64
NCORES = 8
HPC = 4            # heads per core
SC = 512           # q-chunk width
NQC = S // SC      # 4 q-chunks
NKT = S // 128     # 16 k-tiles
F16 = mybir.dt.float16
F32 = mybir.dt.float32
ATTN_SCALE = 1.0 / np.sqrt(HD)

_CACHE = {}


def _build():
    nc = bacc.Bacc("TRN2", target_bir_lowering=False, debug=False, num_devices=NCORES)

    xT_d = nc.declare_dram_parameter("xT", [D, S], F16, isOutput=False)
    wqT_d = nc.declare_dram_parameter("wqT", [D, 256], F16, isOutput=False)
    wkT_d = nc.declare_dram_parameter("wkT", [D, 256], F16, isOutput=False)
    wvT_d = nc.declare_dram_parameter("wvT", [D, 256], F16, isOutput=False)
    woT_d = nc.declare_dram_parameter("woT", [256, D], F16, isOutput=False)
    mask_d = nc.declare_dram_parameter("mask", [4, 128, SC], F16, isOutput=False)
    ones_d = nc.declare_dram_parameter("ones", [128, HPC], F16, isOutput=False)
    y_d = nc.declare_dram_parameter("y", [S, D], F16, isOutput=True)

    with TileContext(nc) as tc:
        with (
            tc.tile_pool(name="static", bufs=1) as st,
            tc.tile_pool(name="ppool", bufs=10) as ppool,
            tc.tile_pool(name="rbpool", bufs=6) as rbpool,
            tc.tile_pool(name="recpool", bufs=4) as recpool,
            tc.tile_pool(name="ystage", bufs=4) as ystage,
            tc.tile_pool(name="psA", bufs=2, space="PSUM") as psA,
            tc.tile_pool(name="psS", bufs=2, space="PSUM") as psS,
            tc.tile_pool(name="psV", bufs=1, space="PSUM") as psV,
        ):
            # ---- input DMAs, ordered for earliest proj(0) start:
            # sync queue:   xT chunk 0, wk, mask, xT chunks 1-3
            # gpsimd queue: wq, wv, wo
            mask = st.tile([128, 4 * SC], F16, name="mask", tag="mask")
            wq = st.tile([128, 2048], F16, name="wq", tag="wq")
            wk = st.tile([128, 2048], F16, name="wk", tag="wk")
            wv = st.tile([128, 2048], F16, name="wv", tag="wv")
            wo = st.tile([128, 2048], F16, name="wo", tag="wo")
            xT = [st.tile([128, S], F16, name=f"xT{k}", tag=f"xT{k}") for k in range(8)]
            for k in range(8):
                nc.sync.dma_start(
                    out=xT[k][:, 0:SC], in_=xT_d[128 * k : 128 * k + 128, 0:SC]
                )
            for wt, wd, eng in (
                (wq, wqT_d, nc.gpsimd),
                (wk, wkT_d, nc.sync),
                (wv, wvT_d, nc.gpsimd),
            ):
                for k in range(8):
                    eng.dma_start(
                        out=wt[:, 256 * k : 256 * k + 256],
                        in_=wd[128 * k : 128 * k + 128, :],
                    )
            for t in range(4):
                nc.sync.dma_start(out=mask[:, SC * t : SC * t + SC], in_=mask_d[t])
            for cc in range(2):
                nc.gpsimd.dma_start(
                    out=wo[:, 1024 * cc : 1024 * cc + 1024],
                    in_=woT_d[128 * cc : 128 * cc + 128, :],
                )
            for n in range(1, NQC):
                for k in range(8):
                    nc.sync.dma_start(
                        out=xT[k][:, SC * n : SC * n + SC],
                        in_=xT_d[128 * k : 128 * k + 128, SC * n : SC * n + SC],
                    )

            QT = [st.tile([128, S], F16, name=f"QT{m}", tag=f"QT{m}") for m in range(2)]
            KT = [st.tile([128, S], F16, name=f"KT{m}", tag=f"KT{m}") for m in range(2)]
            va = [
                st.tile([128, 65 * HPC], F16, name=f"va{i}", tag=f"va{i}")
                for i in range(NKT)
            ]
            outT = [
                st.tile([128, S], F16, name=f"outT{m}", tag=f"outT{m}")
                for m in range(2)
            ]

            def proj_qk_chunk(n):
                for dst, w in ((QT, wq), (KT, wk)):
                    for m in range(2):
                        acc = psA.tile([128, SC], F32, name="acc", tag="acc")
                        for k in range(8):
                            nc.tensor.matmul(
                                acc[:],
                                w[:, 256 * k + 128 * m : 256 * k + 128 * m + 128],
                                xT[k][:, SC * n : SC * n + SC],
                                start=(k == 0),
                                stop=(k == 7),
                            )
                        nc.vector.tensor_copy(dst[m][:, SC * n : SC * n + SC], acc[:])

            def proj_v(i):
                accv = psA.tile([128, 256], F32, name="accv", tag="acc")
                for k in range(8):
                    nc.tensor.matmul(
                        accv[:],
                        xT[k][:, 128 * i : 128 * i + 128],
                        wv[:, 256 * k : 256 * k + 256],
                        start=(k == 0),
                        stop=(k == 7),
                    )
                for h in range(HPC):
                    nc.vector.tensor_copy(
                        va[i][:, 65 * h : 65 * h + 64], accv[:, 64 * h : 64 * h + 64]
                    )
                ones_ap = va[i].rearrange("p (h c) -> p h c", c=65)[:, :, 64]
                nc.sync.dma_start(out=ones_ap, in_=ones_d[:])

            def attn_kloop(jq, hp):
                """Scores+exp+AV for one head pair; av psum tiles returned
                unnormalized (rows 0-63 = out^T, row 64 = denominators)."""
                nkt = 4 * jq + 4  # causal: k-tiles 0 .. 4*jq+3
                av = [
                    psV.tile([65, SC], F32, name=f"av{u}", tag=f"av{u}")
                    for u in range(2)
                ]
                trim = os.environ.get("KV_TRIM", "1") == "1"
                for kt in range(nkt):
                    t = kt - 4 * jq  # >= 0 on diagonal k-tiles
                    # causal trim: the t-th diagonal k-tile is all-masked
                    # for q-columns < 128t; skip them in scores/exp/AV.
                    off = 128 * t if (t > 0 and trim) else 0
                    sp = psS.tile([128, 1024], F32, name="sp", tag="sp")
                    for u, base in ((0, 0), (1, 64)):
                        nc.tensor.matmul(
                            sp[:, 512 * u + off : 512 * u + 512],
                            KT[hp][base : base + 64, 128 * kt : 128 * kt + 128],
                            QT[hp][base : base + 64, SC * jq + off : SC * jq + SC],
                            start=True,
                            stop=True,
                        )
                    pt = ppool.tile([128, 1024], F16, name="pt", tag="pt")
                    if off:
                        sp3 = sp.rearrange("p (u q) -> p u q", u=2)[:, :, off:512]
                        pt3 = pt.rearrange("p (u q) -> p u q", u=2)[:, :, off:512]
                        nc.scalar.activation(
                            pt3,
                            sp3,
                            mybir.ActivationFunctionType.Exp,
                            scale=float(ATTN_SCALE),
                        )
                    else:
                        nc.scalar.activation(
                            pt[:],
                            sp[:],
                            mybir.ActivationFunctionType.Exp,
                            scale=float(ATTN_SCALE),
                        )
                    if t >= 0:  # diagonal k-tile: causal mask
                        for u in range(2):
                            if trim:
                                # only the [128,128] boundary block is
                                # partially masked; its tril pattern is the
                                # first 128 cols of mask pattern 0.
                                blk = slice(512 * u + 128 * t, 512 * u + 128 * t + 128)
                                nc.vector.tensor_mul(
                                    pt[:, blk], pt[:, blk], mask[:, 0:128]
                                )
                            else:
                                sl = slice(512 * u, 512 * u + 512)
                                nc.vector.tensor_mul(
                                    pt[:, sl], pt[:, sl], mask[:, SC * t : SC * t + SC]
                                )
                    for u in range(2):
                        h = 2 * hp + u
                        nc.tensor.matmul(
                            av[u][:, off:SC],
                            va[kt][:, 65 * h : 65 * h + 65],
                            pt[:, 512 * u + off : 512 * u + 512],
                            start=(kt == 0),
                            stop=(kt == nkt - 1),
                        )
                return av

            # normalize columns by softmax denominators (row 64), split in
            # two stages so other work can be emitted between them:
            # stage 1 (DVE only): evacuate av to SBUF (this read is what
            # frees the av psum slots for the next k-loop) and compute the
            # fp16 reciprocal row. stage 2 (1 matmul + 1 DVE mul per head):
            # broadcast 1/den across 64 partitions via a K=1 matmul against
            # a ones row (mask pattern 0 row 0 is all-ones), multiply.
            def attn_norm_s1(av):
                st = []
                for u in range(2):
                    avs = rbpool.tile([65, SC], F32, name="avs", tag="avs")
                    nc.vector.tensor_copy(avs[:], av[u][:])
                    den = recpool.tile([1, SC], F32, name="den", tag="den")
                    nc.vector.tensor_copy(den[:], avs[64:65, :])
                    rec = recpool.tile([1, SC], F32, name="rec", tag="rec")
                    nc.vector.reciprocal_approx_fast(rec[:], den[:])
                    rec16 = recpool.tile([1, SC], F16, name="rec16", tag="rec16")
                    nc.vector.tensor_copy(rec16[:], rec[:])
                    st.append((avs, rec16))
                return st

            def attn_norm_s2(jq, hp, st):
                for u, (avs, rec16) in enumerate(st):
                    rbp = psA.tile([64, SC], F32, name="rbp", tag="acc")
                    nc.tensor.matmul(
                        rbp[:], mask[0:1, 0:64], rec16[:], start=True, stop=True
                    )
                    nc.vector.tensor_mul(
                        outT[hp][64 * u : 64 * u + 64, SC * jq : SC * jq + SC],
                        avs[0:64, :],
                        rbp[:],
                    )

            def attn_normalize(jq, hp, av):
                attn_norm_s2(jq, hp, attn_norm_s1(av))

            def wo_chunk(jq):
                for i in range(4 * jq, 4 * jq + 4):
                    for n in range(2):
                        yp = psA.tile([128, 512], F32, name="yp", tag="acc")
                        for cc in range(2):
                            nc.tensor.matmul(
                                yp[:],
                                outT[cc][:, 128 * i : 128 * i + 128],
                                wo[:, 1024 * cc + 512 * n : 1024 * cc + 512 * n + 512],
                                start=(cc == 0),
                                stop=(cc == 1),
                            )
                        ys = ystage.tile([128, 512], F16, name="ys", tag="ys")
                        nc.vector.tensor_copy(ys[:], yp[:])
                        eng = nc.sync if (i % 2 == 0) else nc.gpsimd
                        eng.dma_start(
                            out=y_d[128 * i : 128 * i + 128, 512 * n : 512 * n + 512],
                            in_=ys[:],
                        )

            # NOTE: projections must not interleave with attention's score/AV
            # psum accumulation groups (nondeterministic hardware corruption,
            # verified repeatedly on HW). Emitting proj(n+1) after chunk n's
            # k-loops but before its hp1 normalize + wo keeps the tensor
            # queue fed through the normalize tail without touching the
            # attention groups.
            # Software pipeline over chunks. Per chunk boundary the emission
            # is: [kloop(n,1)] [proj(n+1)+V] [norm(n,0) stage2]
            # [norm(n,1) stage1] [kloop(n+1,0)] [norm(n,1) stage2] [wo(n)].
            # The next chunk's first k-loop sits BEFORE chunk n's remaining
            # bcast/wo work, so the scalar engine's exp stream resumes as
            # soon as the projections land instead of also waiting out the
            # normalize broadcasts and the output projection. All psum
            # accumulation groups are closed at every insertion point.
            mode = os.environ.get("KV_PIPE", "2")
            if mode == "2":
                proj_qk_chunk(0)
                for i in range(4):
                    proj_v(i)
                av0 = attn_kloop(0, 0)
                for n in range(NQC):
                    s1_0 = attn_norm_s1(av0)
                    av1 = attn_kloop(n, 1)
                    if n + 1 < NQC:
                        proj_qk_chunk(n + 1)
                        for i in range(4 * n + 4, 4 * n + 8):
                            proj_v(i)
                    attn_norm_s2(n, 0, s1_0)
                    s1_1 = attn_norm_s1(av1)
                    if n + 1 < NQC:
                        av0 = attn_kloop(n + 1, 0)
                    attn_norm_s2(n, 1, s1_1)
                    wo_chunk(n)
            elif mode == "1":
                proj_qk_chunk(0)
                for i in range(4):
                    proj_v(i)
                for n in range(NQC):
                    av0 = attn_kloop(n, 0)
                    attn_normalize(n, 0, av0)
                    av1 = attn_kloop(n, 1)
                    if n + 1 < NQC:
                        proj_qk_chunk(n + 1)
                        for i in range(4 * n + 4, 4 * n + 8):
                            proj_v(i)
                    attn_normalize(n, 1, av1)
                    wo_chunk(n)
            else:
                for n in range(NQC):
                    proj_qk_chunk(n)
                    for i in range(4 * n, 4 * n + 4):
                        proj_v(i)
                    av0 = attn_kloop(n, 0)
                    attn_normalize(n, 0, av0)
                    av1 = attn_kloop(n, 1)
                    attn_normalize(n, 1, av1)
                    wo_chunk(n)

    nc.compile()
    return nc


def _masks_np():
    m = np.zeros((4, 128, SC), dtype=np.float16)
    qq = np.arange(SC)[None, :]
    kk = np.arange(128)[:, None]
    for t in range(4):
        m[t] = ((128 * t + kk) <= qq).astype(np.float16)
    return m


def kernel(x, Wq, Wk, Wv, Wo):
    x = np.asarray(x, dtype=np.float32)
    Wq = np.asarray(Wq, dtype=np.float32)
    Wk = np.asarray(Wk, dtype=np.float32)
    Wv = np.asarray(Wv, dtype=np.float32)
    Wo = np.asarray(Wo, dtype=np.float32)

    if "nc" not in _CACHE:
        _CACHE["nc"] = _build()
    nc = _CACHE["nc"]

    masks = _masks_np()
    xT = [np.ascontiguousarray(x[b].T).astype(np.float16) for b in range(B)]
    in_maps = []
    for c in range(NCORES):
        b, g = c // 4, c % 4
        rows = slice(256 * g, 256 * g + 256)
        in_maps.append(
            {
                "xT": xT[b],
                "wqT": np.ascontiguousarray(Wq[rows].T).astype(np.float16),
                "wkT": np.ascontiguousarray(Wk[rows].T).astype(np.float16),
                "wvT": np.ascontiguousarray(Wv[rows].T).astype(np.float16),
                "woT": np.ascontiguousarray(Wo[:, rows].T).astype(np.float16),
                "mask": masks,
                "ones": np.ones((128, HPC), dtype=np.float16),
            }
        )

    trace = False
    if os.environ.get("KERNEL_TRACE") == "1":
        try:
            from trn_agent_boot.trn_boot import _ntff_profile_via_ctypes

            try:
                from antenv.axon_hooks import (
                    get_axon_ntff_profile_hook,
                    set_axon_ntff_profile_hook,
                )
            except ImportError:
                # this image's antenv lacks axon_hooks; provide the
                # 2-function registry bass_utils expects (test-only path)
                import types

                import antenv

                mod = types.ModuleType("antenv.axon_hooks")
                mod._hook = None

                def set_axon_ntff_profile_hook(h, _m=mod):
                    _m._hook = h

                def get_axon_ntff_profile_hook(_m=mod):
                    return _m._hook

                mod.set_axon_ntff_profile_hook = set_axon_ntff_profile_hook
                mod.get_axon_ntff_profile_hook = get_axon_ntff_profile_hook
                sys.modules["antenv.axon_hooks"] = mod
                antenv.axon_hooks = mod

            if get_axon_ntff_profile_hook() is None:
                set_axon_ntff_profile_hook(
                    _ntff_profile_via_ctypes("/opt/axon/libaxon_pjrt.so")
                )
            trace = True
        except Exception:
            trace = False

    res = run_bass_kernel_spmd(nc, in_maps, core_ids=list(range(NCORES)), trace=trace)
    _CACHE["exec_time_ns"] = res.exec_time_ns
    _CACHE["res"] = res
    y = np.zeros((B, S, D), dtype=np.float32)
    for c in range(NCORES):
        y[c // 4] += res.results[c]["y"].astype(np.float32)
    return y


# revision 25
# speedup vs baseline: 1.2241x; 1.0117x over previous
"""Multi-head self-attention (B=2, S=2048, D=1024, H=16, causal) on 8 NeuronCores.

Sharding: 32 (batch, head) instances -> 4 heads of one batch per core
(cores 0-3: batch 0, cores 4-7: batch 1; core c owns heads 4*(c%4) .. +3).
Wq/Wk/Wv are split by rows (head dims), Wo by columns; each core computes a
partial y[b] = attn_out_heads @ Wo_cols.T and the host sums the 4 partials
per batch at gather time (tensor-parallel reduce).

Per-core kernel. All matmuls fp16 x fp16 -> fp32 psum (fp16 stationary
operands get fast, reorder-hidden LDWEIGHTS; float32r would force a fused
half-rate weight load serialized with every matmul). No on-device transposes:
  QT[256,2048] = wqT.T @ xT        (head-pair tiles: rows 0-63 / 64-127)
  KT likewise; V[2048,256] natural (lhsT = xT chunks), augmented with a
  ones column per head -> va tiles [128, 4*65].
  Scores computed transposed, blockwise [k-tile 128, q-chunk 512]:
      S^T = KT_h.T @ QT_h   -- two heads row-packed (contraction d=64 at
      partition bases 0 / 64). Both heads of a k-tile share one [128,1024]
      psum tile (bufs=2 -> exp overlaps the next k-tile's score matmuls);
      one Exp (scale=1/8) per tile on ScalarE -> P fp16 in SBUF.
  Causal masking only on diagonal k-tiles via precomputed 0/1 mask multiply.
  AV: lhsT = [V_h | 1] fp16 [k,65], rhs = P [k,512] -> psum [65,512]
      accumulated over k-tiles = unnormalized out^T (rows 0-63) + softmax
      denominators (row 64). Normalize columns via reciprocal_approx_fast +
      matmul partition broadcast + DVE multiply -> out_headsT [256,2048] fp16.
  y = out_headsT.T @ woT -> [2048, 1024] fp16 partial, DMA'd out
      (host accumulates partials in fp32).

Pipeline over q-chunks. The next chunk's QKV projections are emitted after
chunk n's two attention k-loops but before hp1's normalize + output
projection, so the tensor engine stays busy (and the PE clock-gate stays
warm) through the normalize tail. Projections are never interleaved with
attention's score/AV psum accumulation groups (nondeterministic hardware
corruption when they are; verified on HW).
"""
import os
import sys

sys.path.insert(0, "/opt/trn_rl_repo")

import numpy as np

import concourse.bass as bass  # noqa: F401
import concourse.mybir as mybir
from concourse import bacc
from concourse.tile import TileContext
from concourse.bass_utils import run_bass_kernel_spmd

B, S, D = 2, 2048, 1024
H, HD = 16, 64
NCORES = 8
HPC = 4            # heads per core
SC = 512           # q-chunk width
NQC = S // SC      # 4 q-chunks
NKT = S // 128     # 16 k-tiles
F16 = mybir.dt.float16
F32 = mybir.dt.float32
ATTN_SCALE = 1.0 / np.sqrt(HD)

_CACHE = {}


def _build():
    nc = bacc.Bacc("TRN2", target_bir_lowering=False, debug=False, num_devices=NCORES)

    xT_d = nc.declare_dram_parameter("xT", [D, S], F16, isOutput=False)
    wqT_d = nc.declare_dram_parameter("wqT", [D, 256], F16, isOutput=False)
    wkT_d = nc.declare_dram_parameter("wkT", [D, 256], F16, isOutput=False)
    wvT_d = nc.declare_dram_parameter("wvT", [D, 256], F16, isOutput=False)
    woT_d = nc.declare_dram_parameter("woT", [256, D], F16, isOutput=False)
    mask_d = nc.declare_dram_parameter("mask", [4, 128, SC], F16, isOutput=False)
    ones_d = nc.declare_dram_parameter("ones", [128, HPC], F16, isOutput=False)
    y_d = nc.declare_dram_parameter("y", [S, D], F16, isOutput=True)

    with TileContext(nc) as tc:
        with (
            tc.tile_pool(name="static", bufs=1) as st,
            tc.tile_pool(name="ppool", bufs=10) as ppool,
            tc.tile_pool(name="rbpool", bufs=6) as rbpool,
            tc.tile_pool(name="recpool", bufs=4) as recpool,
            tc.tile_pool(name="ystage", bufs=4) as ystage,
            tc.tile_pool(name="psA", bufs=2, space="PSUM") as psA,
            tc.tile_pool(name="psS", bufs=2, space="PSUM") as psS,
            tc.tile_pool(name="psV", bufs=1, space="PSUM") as psV,
        ):
            # ---- input DMAs, ordered for earliest proj(0) start:
            # sync queue:   xT chunk 0, wk, mask, xT chunks 1-3
            # gpsimd queue: wq, wv, wo
            mask = st.tile([128, 4 * SC], F16, name="mask", tag="mask")
            wq = st.tile([128, 2048], F16, name="wq", tag="wq")
            wk = st.tile([128, 2048], F16, name="wk", tag="wk")
            wv = st.tile([128, 2048], F16, name="wv", tag="wv")
            wo = st.tile([128, 2048], F16, name="wo", tag="wo")
            xT = [st.tile([128, S], F16, name=f"xT{k}", tag=f"xT{k}") for k in range(8)]
            for k in range(8):
                nc.sync.dma_start(
                    out=xT[k][:, 0:SC], in_=xT_d[128 * k : 128 * k + 128, 0:SC]
                )
            for wt, wd, eng in (
                (wq, wqT_d, nc.gpsimd),
                (wk, wkT_d, nc.sync),
                (wv, wvT_d, nc.gpsimd),
            ):
                for k in range(8):
                    eng.dma_start(
                        out=wt[:, 256 * k : 256 * k + 256],
                        in_=wd[128 * k : 128 * k + 128, :],
                    )
            for t in range(4):
                nc.sync.dma_start(out=mask[:, SC * t : SC * t + SC], in_=mask_d[t])
            for cc in range(2):
                nc.gpsimd.dma_start(
                    out=wo[:, 1024 * cc : 1024 * cc + 1024],
                    in_=woT_d[128 * cc : 128 * cc + 128, :],
                )
            for n in range(1, NQC):
                for k in range(8):
                    nc.sync.dma_start(
                        out=xT[k][:, SC * n : SC * n + SC],
                        in_=xT_d[128 * k : 128 * k + 128, SC * n : SC * n + SC],
                    )

            QT = [st.tile([128, S], F16, name=f"QT{m}", tag=f"QT{m}") for m in range(2)]
            KT = [st.tile([128, S], F16, name=f"KT{m}", tag=f"KT{m}") for m in range(2)]
            va = [
                st.tile([128, 65 * HPC], F16, name=f"va{i}", tag=f"va{i}")
                for i in range(NKT)
            ]
            outT = [
                st.tile([128, S], F16, name=f"outT{m}", tag=f"outT{m}")
                for m in range(2)
            ]

            def proj_qk_chunk(n):
                for dst, w in ((QT, wq), (KT, wk)):
                    for m in range(2):
                        acc = psA.tile([128, SC], F32, name="acc", tag="acc")
                        for k in range(8):
                            nc.tensor.matmul(
                                acc[:],
                                w[:, 256 * k + 128 * m : 256 * k + 128 * m + 128],
                                xT[k][:, SC * n : SC * n + SC],
                                start=(k == 0),
                                stop=(k == 7),
                            )
                        nc.vector.tensor_copy(dst[m][:, SC * n : SC * n + SC], acc[:])

            def proj_v(i):
                accv = psA.tile([128, 256], F32, name="accv", tag="acc")
                for k in range(8):
                    nc.tensor.matmul(
                        accv[:],
                        xT[k][:, 128 * i : 128 * i + 128],
                        wv[:, 256 * k : 256 * k + 256],
                        start=(k == 0),
                        stop=(k == 7),
                    )
                for h in range(HPC):
                    nc.vector.tensor_copy(
                        va[i][:, 65 * h : 65 * h + 64], accv[:, 64 * h : 64 * h + 64]
                    )
                ones_ap = va[i].rearrange("p (h c) -> p h c", c=65)[:, :, 64]
                nc.sync.dma_start(out=ones_ap, in_=ones_d[:])

            def attn_kloop(jq, hp):
                """Scores+exp+AV for one head pair; av psum tiles returned
                unnormalized (rows 0-63 = out^T, row 64 = denominators)."""
                nkt = 4 * jq + 4  # causal: k-tiles 0 .. 4*jq+3
                av = [
                    psV.tile([65, SC], F32, name=f"av{u}", tag=f"av{u}")
                    for u in range(2)
                ]
                trim = os.environ.get("KV_TRIM", "1") == "1"
                for kt in range(nkt):
                    t = kt - 4 * jq  # >= 0 on diagonal k-tiles
                    # causal trim: the t-th diagonal k-tile is all-masked
                    # for q-columns < 128t; skip them in scores/exp/AV.
                    off = 128 * t if (t > 0 and trim) else 0
                    sp = psS.tile([128, 1024], F32, name="sp", tag="sp")
                    for u, base in ((0, 0), (1, 64)):
                        nc.tensor.matmul(
                            sp[:, 512 * u + off : 512 * u + 512],
                            KT[hp][base : base + 64, 128 * kt : 128 * kt + 128],
                            QT[hp][base : base + 64, SC * jq + off : SC * jq + SC],
                            start=True,
                            stop=True,
                        )
                    pt = ppool.tile([128, 1024], F16, name="pt", tag="pt")
                    if off:
                        sp3 = sp.rearrange("p (u q) -> p u q", u=2)[:, :, off:512]
                        pt3 = pt.rearrange("p (u q) -> p u q", u=2)[:, :, off:512]
                        nc.scalar.activation(
                            pt3,
                            sp3,
                            mybir.ActivationFunctionType.Exp,
                            scale=float(ATTN_SCALE),
                        )
                    else:
                        nc.scalar.activation(
                            pt[:],
                            sp[:],
                            mybir.ActivationFunctionType.Exp,
                            scale=float(ATTN_SCALE),
                        )
                    if t >= 0:  # diagonal k-tile: causal mask
                        for u in range(2):
                            if trim:
                                # only the [128,128] boundary block is
                                # partially masked; its tril pattern is the
                                # first 128 cols of mask pattern 0.
                                blk = slice(512 * u + 128 * t, 512 * u + 128 * t + 128)
                                nc.vector.tensor_mul(
                                    pt[:, blk], pt[:, blk], mask[:, 0:128]
                                )
                            else:
                                sl = slice(512 * u, 512 * u + 512)
                                nc.vector.tensor_mul(
                                    pt[:, sl], pt[:, sl], mask[:, SC * t : SC * t + SC]
                                )
                    for u in range(2):
                        h = 2 * hp + u
                        nc.tensor.matmul(
                            av[u][:, off:SC],
                            va[kt][:, 65 * h : 65 * h + 65],
                            pt[:, 512 * u + off : 512 * u + 512],
                            start=(kt == 0),
                            stop=(kt == nkt - 1),
                        )
                return av

            # normalize columns by softmax denominators (row 64), split in
            # two stages so other work can be emitted between them:
            # stage 1 (DVE only): evacuate av to SBUF (this read is what
            # frees the av psum slots for the next k-loop) and compute the
            # fp16 reciprocal row. stage 2 (1 matmul + 1 DVE mul per head):
            # broadcast 1/den across 64 partitions via a K=1 matmul against
            # a ones row (mask pattern 0 row 0 is all-ones), multiply.
            def attn_norm_s1(av):
                # evacuate both heads first: these two copies are what free
                # the av psum slots, and the next k-loop's mask multiplies
                # queue behind them on the DVE -- keep the (longer)
                # reciprocal chains after both copies.
                avss = []
                for u in range(2):
                    avs = rbpool.tile([65, SC], F32, name="avs", tag="avs")
                    nc.vector.tensor_copy(avs[:], av[u][:])
                    avss.append(avs)
                st = []
                for u in range(2):
                    den = recpool.tile([1, SC], F32, name="den", tag="den")
                    nc.vector.tensor_copy(den[:], avss[u][64:65, :])
                    rec = recpool.tile([1, SC], F32, name="rec", tag="rec")
                    nc.vector.reciprocal_approx_fast(rec[:], den[:])
                    rec16 = recpool.tile([1, SC], F16, name="rec16", tag="rec16")
                    nc.vector.tensor_copy(rec16[:], rec[:])
                    st.append((avss[u], rec16))
                return st

            def attn_norm_s2(jq, hp, st):
                for u, (avs, rec16) in enumerate(st):
                    rbp = psA.tile([64, SC], F32, name="rbp", tag="acc")
                    nc.tensor.matmul(
                        rbp[:], mask[0:1, 0:64], rec16[:], start=True, stop=True
                    )
                    nc.vector.tensor_mul(
                        outT[hp][64 * u : 64 * u + 64, SC * jq : SC * jq + SC],
                        avs[0:64, :],
                        rbp[:],
                    )

            def attn_normalize(jq, hp, av):
                attn_norm_s2(jq, hp, attn_norm_s1(av))

            def wo_chunk(jq):
                for i in range(4 * jq, 4 * jq + 4):
                    for n in range(2):
                        yp = psA.tile([128, 512], F32, name="yp", tag="acc")
                        for cc in range(2):
                            nc.tensor.matmul(
                                yp[:],
                                outT[cc][:, 128 * i : 128 * i + 128],
                                wo[:, 1024 * cc + 512 * n : 1024 * cc + 512 * n + 512],
                                start=(cc == 0),
                                stop=(cc == 1),
                            )
                        ys = ystage.tile([128, 512], F16, name="ys", tag="ys")
                        nc.vector.tensor_copy(ys[:], yp[:])
                        if jq == NQC - 1:
                            # final chunk: also use the (idle) scalar queue
                            # so the last transfers drain in parallel
                            eng = (nc.sync, nc.gpsimd, nc.scalar)[(2 * i + n) % 3]
                        else:
                            eng = nc.sync if (i % 2 == 0) else nc.gpsimd
                        eng.dma_start(
                            out=y_d[128 * i : 128 * i + 128, 512 * n : 512 * n + 512],
                            in_=ys[:],
                        )

            # NOTE: projections must not interleave with attention's score/AV
            # psum accumulation groups (nondeterministic hardware corruption,
            # verified repeatedly on HW). Emitting proj(n+1) after chunk n's
            # k-loops but before its hp1 normalize + wo keeps the tensor
            # queue fed through the normalize tail without touching the
            # attention groups.
            # Software pipeline over chunks. Per chunk boundary the emission
            # is: [kloop(n,1)] [proj(n+1)+V] [norm(n,0) stage2]
            # [norm(n,1) stage1] [kloop(n+1,0)] [norm(n,1) stage2] [wo(n)].
            # The next chunk's first k-loop sits BEFORE chunk n's remaining
            # bcast/wo work, so the scalar engine's exp stream resumes as
            # soon as the projections land instead of also waiting out the
            # normalize broadcasts and the output projection. All psum
            # accumulation groups are closed at every insertion point.
            mode = os.environ.get("KV_PIPE", "2")
            if mode == "2":
                proj_qk_chunk(0)
                for i in range(4):
                    proj_v(i)
                av0 = attn_kloop(0, 0)
                for n in range(NQC):
                    s1_0 = attn_norm_s1(av0)
                    av1 = attn_kloop(n, 1)
                    if n + 1 < NQC:
                        proj_qk_chunk(n + 1)
                        for i in range(4 * n + 4, 4 * n + 8):
                            proj_v(i)
                    attn_norm_s2(n, 0, s1_0)
                    s1_1 = attn_norm_s1(av1)
                    if n + 1 < NQC:
                        av0 = attn_kloop(n + 1, 0)
                    attn_norm_s2(n, 1, s1_1)
                    wo_chunk(n)
            elif mode == "1":
                proj_qk_chunk(0)
                for i in range(4):
                    proj_v(i)
                for n in range(NQC):
                    av0 = attn_kloop(n, 0)
                    attn_normalize(n, 0, av0)
                    av1 = attn_kloop(n, 1)
                    if n + 1 < NQC:
                        proj_qk_chunk(n + 1)
                        for i in range(4 * n + 4, 4 * n + 8):
                            proj_v(i)
                    attn_normalize(n, 1, av1)
                    wo_chunk(n)
            else:
                for n in range(NQC):
                    proj_qk_chunk(n)
                    for i in range(4 * n, 4 * n + 4):
                        proj_v(i)
                    av0 = attn_kloop(n, 0)
                    attn_normalize(n, 0, av0)
                    av1 = attn_kloop(n, 1)
                    attn_normalize(n, 1, av1)
                    wo_chunk(n)

    nc.compile()
    return nc


def _masks_np():
    m = np.zeros((4, 128, SC), dtype=np.float16)
    qq = np.arange(SC)[None, :]
    kk = np.arange(128)[:, None]
    for t in range(4):
        m[t] = ((128 * t + kk) <= qq).astype(np.float16)
    return m


def kernel(x, Wq, Wk, Wv, Wo):
    x = np.asarray(x, dtype=np.float32)
    Wq = np.asarray(Wq, dtype=np.float32)
    Wk = np.asarray(Wk, dtype=np.float32)
    Wv = np.asarray(Wv, dtype=np.float32)
    Wo = np.asarray(Wo, dtype=np.float32)

    if "nc" not in _CACHE:
        _CACHE["nc"] = _build()
    nc = _CACHE["nc"]

    masks = _masks_np()
    xT = [np.ascontiguousarray(x[b].T).astype(np.float16) for b in range(B)]
    in_maps = []
    for c in range(NCORES):
        b, g = c // 4, c % 4
        rows = slice(256 * g, 256 * g + 256)
        in_maps.append(
            {
                "xT": xT[b],
                "wqT": np.ascontiguousarray(Wq[rows].T).astype(np.float16),
                "wkT": np.ascontiguousarray(Wk[rows].T).astype(np.float16),
                "wvT": np.ascontiguousarray(Wv[rows].T).astype(np.float16),
                "woT": np.ascontiguousarray(Wo[:, rows].T).astype(np.float16),
                "mask": masks,
                "ones": np.ones((128, HPC), dtype=np.float16),
            }
        )

    trace = False
    if os.environ.get("KERNEL_TRACE") == "1":
        try:
            from trn_agent_boot.trn_boot import _ntff_profile_via_ctypes

            try:
                from antenv.axon_hooks import (
                    get_axon_ntff_profile_hook,
                    set_axon_ntff_profile_hook,
                )
            except ImportError:
                # this image's antenv lacks axon_hooks; provide the
                # 2-function registry bass_utils expects (test-only path)
                import types

                import antenv

                mod = types.ModuleType("antenv.axon_hooks")
                mod._hook = None

                def set_axon_ntff_profile_hook(h, _m=mod):
                    _m._hook = h

                def get_axon_ntff_profile_hook(_m=mod):
                    return _m._hook

                mod.set_axon_ntff_profile_hook = set_axon_ntff_profile_hook
                mod.get_axon_ntff_profile_hook = get_axon_ntff_profile_hook
                sys.modules["antenv.axon_hooks"] = mod
                antenv.axon_hooks = mod

            if get_axon_ntff_profile_hook() is None:
                set_axon_ntff_profile_hook(
                    _ntff_profile_via_ctypes("/opt/axon/libaxon_pjrt.so")
                )
            trace = True
        except Exception:
            trace = False

    res = run_bass_kernel_spmd(nc, in_maps, core_ids=list(range(NCORES)), trace=trace)
    _CACHE["exec_time_ns"] = res.exec_time_ns
    _CACHE["res"] = res
    y = np.zeros((B, S, D), dtype=np.float32)
    for c in range(NCORES):
        y[c // 4] += res.results[c]["y"].astype(np.float32)
    return y


# revision 26
# speedup vs baseline: 1.2295x; 1.0044x over previous
"""Multi-head self-attention (B=2, S=2048, D=1024, H=16, causal) on 8 NeuronCores.

Sharding: 32 (batch, head) instances -> 4 heads of one batch per core
(cores 0-3: batch 0, cores 4-7: batch 1; core c owns heads 4*(c%4) .. +3).
Wq/Wk/Wv are split by rows (head dims), Wo by columns; each core computes a
partial y[b] = attn_out_heads @ Wo_cols.T and the host sums the 4 partials
per batch at gather time (tensor-parallel reduce).

Per-core kernel. All matmuls fp16 x fp16 -> fp32 psum (fp16 stationary
operands get fast, reorder-hidden LDWEIGHTS; float32r would force a fused
half-rate weight load serialized with every matmul). No on-device transposes:
  QT[256,2048] = wqT.T @ xT        (head-pair tiles: rows 0-63 / 64-127)
  KT likewise; V[2048,256] natural (lhsT = xT chunks), augmented with a
  ones column per head -> va tiles [128, 4*65].
  Scores computed transposed, blockwise [k-tile 128, q-chunk 512]:
      S^T = KT_h.T @ QT_h   -- two heads row-packed (contraction d=64 at
      partition bases 0 / 64). Both heads of a k-tile share one [128,1024]
      psum tile (bufs=2 -> exp overlaps the next k-tile's score matmuls);
      one Exp (scale=1/8) per tile on ScalarE -> P fp16 in SBUF.
  Causal masking only on diagonal k-tiles via precomputed 0/1 mask multiply.
  AV: lhsT = [V_h | 1] fp16 [k,65], rhs = P [k,512] -> psum [65,512]
      accumulated over k-tiles = unnormalized out^T (rows 0-63) + softmax
      denominators (row 64). Normalize columns via reciprocal_approx_fast +
      matmul partition broadcast + DVE multiply -> out_headsT [256,2048] fp16.
  y = out_headsT.T @ woT -> [2048, 1024] fp16 partial, DMA'd out
      (host accumulates partials in fp32).

Pipeline over q-chunks. The next chunk's QKV projections are emitted after
chunk n's two attention k-loops but before hp1's normalize + output
projection, so the tensor engine stays busy (and the PE clock-gate stays
warm) through the normalize tail. Projections are never interleaved with
attention's score/AV psum accumulation groups (nondeterministic hardware
corruption when they are; verified on HW).
"""
import os
import sys

sys.path.insert(0, "/opt/trn_rl_repo")

import numpy as np

import concourse.bass as bass  # noqa: F401
import concourse.mybir as mybir
from concourse import bacc
from concourse.tile import TileContext
from concourse.bass_utils import run_bass_kernel_spmd

B, S, D = 2, 2048, 1024
H, HD = 16, 64
NCORES = 8
HPC = 4            # heads per core
SC = 512           # q-chunk width
NQC = S // SC      # 4 q-chunks
NKT = S // 128     # 16 k-tiles
F16 = mybir.dt.float16
F32 = mybir.dt.float32
ATTN_SCALE = 1.0 / np.sqrt(HD)

_CACHE = {}


def _build():
    nc = bacc.Bacc("TRN2", target_bir_lowering=False, debug=False, num_devices=NCORES)

    xT_d = nc.declare_dram_parameter("xT", [D, S], F16, isOutput=False)
    wqT_d = nc.declare_dram_parameter("wqT", [D, 256], F16, isOutput=False)
    wkT_d = nc.declare_dram_parameter("wkT", [D, 256], F16, isOutput=False)
    wvT_d = nc.declare_dram_parameter("wvT", [D, 256], F16, isOutput=False)
    woT_d = nc.declare_dram_parameter("woT", [256, D], F16, isOutput=False)
    mask_d = nc.declare_dram_parameter("mask", [4, 128, SC], F16, isOutput=False)
    ones_d = nc.declare_dram_parameter("ones", [128, HPC], F16, isOutput=False)
    y_d = nc.declare_dram_parameter("y", [S, D], F16, isOutput=True)

    with TileContext(nc) as tc:
        with (
            tc.tile_pool(name="static", bufs=1) as st,
            tc.tile_pool(name="ppool", bufs=10) as ppool,
            tc.tile_pool(name="rbpool", bufs=6) as rbpool,
            tc.tile_pool(name="recpool", bufs=4) as recpool,
            tc.tile_pool(name="ystage", bufs=4) as ystage,
            tc.tile_pool(name="psA", bufs=2, space="PSUM") as psA,
            tc.tile_pool(name="psS", bufs=2, space="PSUM") as psS,
            tc.tile_pool(name="psV", bufs=1, space="PSUM") as psV,
        ):
            # ---- input DMAs, ordered for earliest proj(0) start:
            # sync queue:   xT chunk 0, wk, mask, xT chunks 1-3
            # gpsimd queue: wq, wv, wo
            mask = st.tile([128, 4 * SC], F16, name="mask", tag="mask")
            wq = st.tile([128, 2048], F16, name="wq", tag="wq")
            wk = st.tile([128, 2048], F16, name="wk", tag="wk")
            wv = st.tile([128, 2048], F16, name="wv", tag="wv")
            wo = st.tile([128, 2048], F16, name="wo", tag="wo")
            xT = [st.tile([128, S], F16, name=f"xT{k}", tag=f"xT{k}") for k in range(8)]
            for k in range(8):
                nc.sync.dma_start(
                    out=xT[k][:, 0:SC], in_=xT_d[128 * k : 128 * k + 128, 0:SC]
                )
            for wt, wd, eng in (
                (wq, wqT_d, nc.gpsimd),
                (wk, wkT_d, nc.sync),
                (wv, wvT_d, nc.gpsimd),
            ):
                for k in range(8):
                    eng.dma_start(
                        out=wt[:, 256 * k : 256 * k + 256],
                        in_=wd[128 * k : 128 * k + 128, :],
                    )
            for t in range(4):
                nc.sync.dma_start(out=mask[:, SC * t : SC * t + SC], in_=mask_d[t])
            for cc in range(2):
                nc.gpsimd.dma_start(
                    out=wo[:, 1024 * cc : 1024 * cc + 1024],
                    in_=woT_d[128 * cc : 128 * cc + 128, :],
                )
            for n in range(1, NQC):
                for k in range(8):
                    nc.sync.dma_start(
                        out=xT[k][:, SC * n : SC * n + SC],
                        in_=xT_d[128 * k : 128 * k + 128, SC * n : SC * n + SC],
                    )

            # throwaway exp so the ~2.7us exp_and_others ACT table load
            # happens during the projection phase instead of serially in
            # front of chunk 0's first real exp. Output is never read.
            expwarm = st.tile([1, 8], F16, name="expwarm", tag="expwarm")
            nc.scalar.activation(
                expwarm[:],
                mask[0:1, 0:8],
                mybir.ActivationFunctionType.Exp,
                scale=1.0,
            )

            QT = [st.tile([128, S], F16, name=f"QT{m}", tag=f"QT{m}") for m in range(2)]
            KT = [st.tile([128, S], F16, name=f"KT{m}", tag=f"KT{m}") for m in range(2)]
            va = [
                st.tile([128, 65 * HPC], F16, name=f"va{i}", tag=f"va{i}")
                for i in range(NKT)
            ]
            outT = [
                st.tile([128, S], F16, name=f"outT{m}", tag=f"outT{m}")
                for m in range(2)
            ]

            def proj_qk_chunk(n):
                for dst, w in ((QT, wq), (KT, wk)):
                    for m in range(2):
                        acc = psA.tile([128, SC], F32, name="acc", tag="acc")
                        for k in range(8):
                            nc.tensor.matmul(
                                acc[:],
                                w[:, 256 * k + 128 * m : 256 * k + 128 * m + 128],
                                xT[k][:, SC * n : SC * n + SC],
                                start=(k == 0),
                                stop=(k == 7),
                            )
                        nc.vector.tensor_copy(dst[m][:, SC * n : SC * n + SC], acc[:])

            def proj_v(i):
                accv = psA.tile([128, 256], F32, name="accv", tag="acc")
                for k in range(8):
                    nc.tensor.matmul(
                        accv[:],
                        xT[k][:, 128 * i : 128 * i + 128],
                        wv[:, 256 * k : 256 * k + 256],
                        start=(k == 0),
                        stop=(k == 7),
                    )
                for h in range(HPC):
                    nc.vector.tensor_copy(
                        va[i][:, 65 * h : 65 * h + 64], accv[:, 64 * h : 64 * h + 64]
                    )
                ones_ap = va[i].rearrange("p (h c) -> p h c", c=65)[:, :, 64]
                nc.sync.dma_start(out=ones_ap, in_=ones_d[:])

            def attn_kloop(jq, hp):
                """Scores+exp+AV for one head pair; av psum tiles returned
                unnormalized (rows 0-63 = out^T, row 64 = denominators)."""
                nkt = 4 * jq + 4  # causal: k-tiles 0 .. 4*jq+3
                av = [
                    psV.tile([65, SC], F32, name=f"av{u}", tag=f"av{u}")
                    for u in range(2)
                ]
                trim = os.environ.get("KV_TRIM", "1") == "1"
                for kt in range(nkt):
                    t = kt - 4 * jq  # >= 0 on diagonal k-tiles
                    # causal trim: the t-th diagonal k-tile is all-masked
                    # for q-columns < 128t; skip them in scores/exp/AV.
                    off = 128 * t if (t > 0 and trim) else 0
                    sp = psS.tile([128, 1024], F32, name="sp", tag="sp")
                    for u, base in ((0, 0), (1, 64)):
                        nc.tensor.matmul(
                            sp[:, 512 * u + off : 512 * u + 512],
                            KT[hp][base : base + 64, 128 * kt : 128 * kt + 128],
                            QT[hp][base : base + 64, SC * jq + off : SC * jq + SC],
                            start=True,
                            stop=True,
                        )
                    pt = ppool.tile([128, 1024], F16, name="pt", tag="pt")
                    if off:
                        sp3 = sp.rearrange("p (u q) -> p u q", u=2)[:, :, off:512]
                        pt3 = pt.rearrange("p (u q) -> p u q", u=2)[:, :, off:512]
                        nc.scalar.activation(
                            pt3,
                            sp3,
                            mybir.ActivationFunctionType.Exp,
                            scale=float(ATTN_SCALE),
                        )
                    else:
                        nc.scalar.activation(
                            pt[:],
                            sp[:],
                            mybir.ActivationFunctionType.Exp,
                            scale=float(ATTN_SCALE),
                        )
                    if t >= 0:  # diagonal k-tile: causal mask
                        for u in range(2):
                            if trim:
                                # only the [128,128] boundary block is
                                # partially masked; its tril pattern is the
                                # first 128 cols of mask pattern 0.
                                blk = slice(512 * u + 128 * t, 512 * u + 128 * t + 128)
                                nc.vector.tensor_mul(
                                    pt[:, blk], pt[:, blk], mask[:, 0:128]
                                )
                            else:
                                sl = slice(512 * u, 512 * u + 512)
                                nc.vector.tensor_mul(
                                    pt[:, sl], pt[:, sl], mask[:, SC * t : SC * t + SC]
                                )
                    for u in range(2):
                        h = 2 * hp + u
                        nc.tensor.matmul(
                            av[u][:, off:SC],
                            va[kt][:, 65 * h : 65 * h + 65],
                            pt[:, 512 * u + off : 512 * u + 512],
                            start=(kt == 0),
                            stop=(kt == nkt - 1),
                        )
                return av

            # normalize columns by softmax denominators (row 64), split in
            # two stages so other work can be emitted between them:
            # stage 1 (DVE only): evacuate av to SBUF (this read is what
            # frees the av psum slots for the next k-loop) and compute the
            # fp16 reciprocal row. stage 2 (1 matmul + 1 DVE mul per head):
            # broadcast 1/den across 64 partitions via a K=1 matmul against
            # a ones row (mask pattern 0 row 0 is all-ones), multiply.
            def attn_norm_s1(av):
                # evacuate both heads first: these two copies are what free
                # the av psum slots, and the next k-loop's mask multiplies
                # queue behind them on the DVE -- keep the (longer)
                # reciprocal chains after both copies.
                avss = []
                for u in range(2):
                    avs = rbpool.tile([65, SC], F32, name="avs", tag="avs")
                    nc.vector.tensor_copy(avs[:], av[u][:])
                    avss.append(avs)
                st = []
                for u in range(2):
                    den = recpool.tile([1, SC], F32, name="den", tag="den")
                    nc.vector.tensor_copy(den[:], avss[u][64:65, :])
                    rec = recpool.tile([1, SC], F32, name="rec", tag="rec")
                    nc.vector.reciprocal_approx_fast(rec[:], den[:])
                    rec16 = recpool.tile([1, SC], F16, name="rec16", tag="rec16")
                    nc.vector.tensor_copy(rec16[:], rec[:])
                    st.append((avss[u], rec16))
                return st

            def attn_norm_s2(jq, hp, st):
                for u, (avs, rec16) in enumerate(st):
                    rbp = psA.tile([64, SC], F32, name="rbp", tag="acc")
                    nc.tensor.matmul(
                        rbp[:], mask[0:1, 0:64], rec16[:], start=True, stop=True
                    )
                    nc.vector.tensor_mul(
                        outT[hp][64 * u : 64 * u + 64, SC * jq : SC * jq + SC],
                        avs[0:64, :],
                        rbp[:],
                    )

            def attn_normalize(jq, hp, av):
                attn_norm_s2(jq, hp, attn_norm_s1(av))

            def wo_chunk(jq):
                for i in range(4 * jq, 4 * jq + 4):
                    for n in range(2):
                        yp = psA.tile([128, 512], F32, name="yp", tag="acc")
                        for cc in range(2):
                            nc.tensor.matmul(
                                yp[:],
                                outT[cc][:, 128 * i : 128 * i + 128],
                                wo[:, 1024 * cc + 512 * n : 1024 * cc + 512 * n + 512],
                                start=(cc == 0),
                                stop=(cc == 1),
                            )
                        ys = ystage.tile([128, 512], F16, name="ys", tag="ys")
                        nc.vector.tensor_copy(ys[:], yp[:])
                        if jq == NQC - 1:
                            # final chunk: also use the (idle) scalar queue
                            # so the last transfers drain in parallel
                            eng = (nc.sync, nc.gpsimd, nc.scalar)[(2 * i + n) % 3]
                        else:
                            eng = nc.sync if (i % 2 == 0) else nc.gpsimd
                        eng.dma_start(
                            out=y_d[128 * i : 128 * i + 128, 512 * n : 512 * n + 512],
                            in_=ys[:],
                        )

            # NOTE: projections must not interleave with attention's score/AV
            # psum accumulation groups (nondeterministic hardware corruption,
            # verified repeatedly on HW). Emitting proj(n+1) after chunk n's
            # k-loops but before its hp1 normalize + wo keeps the tensor
            # queue fed through the normalize tail without touching the
            # attention groups.
            # Software pipeline over chunks. Per chunk boundary the emission
            # is: [kloop(n,1)] [proj(n+1)+V] [norm(n,0) stage2]
            # [norm(n,1) stage1] [kloop(n+1,0)] [norm(n,1) stage2] [wo(n)].
            # The next chunk's first k-loop sits BEFORE chunk n's remaining
            # bcast/wo work, so the scalar engine's exp stream resumes as
            # soon as the projections land instead of also waiting out the
            # normalize broadcasts and the output projection. All psum
            # accumulation groups are closed at every insertion point.
            mode = os.environ.get("KV_PIPE", "2")
            if mode == "2":
                proj_qk_chunk(0)
                for i in range(4):
                    proj_v(i)
                av0 = attn_kloop(0, 0)
                for n in range(NQC):
                    s1_0 = attn_norm_s1(av0)
                    av1 = attn_kloop(n, 1)
                    if n + 1 < NQC:
                        proj_qk_chunk(n + 1)
                        for i in range(4 * n + 4, 4 * n + 8):
                            proj_v(i)
                    attn_norm_s2(n, 0, s1_0)
                    s1_1 = attn_norm_s1(av1)
                    if n + 1 < NQC:
                        av0 = attn_kloop(n + 1, 0)
                    attn_norm_s2(n, 1, s1_1)
                    wo_chunk(n)
            elif mode == "1":
                proj_qk_chunk(0)
                for i in range(4):
                    proj_v(i)
                for n in range(NQC):
                    av0 = attn_kloop(n, 0)
                    attn_normalize(n, 0, av0)
                    av1 = attn_kloop(n, 1)
                    if n + 1 < NQC:
                        proj_qk_chunk(n + 1)
                        for i in range(4 * n + 4, 4 * n + 8):
                            proj_v(i)
                    attn_normalize(n, 1, av1)
                    wo_chunk(n)
            else:
                for n in range(NQC):
                    proj_qk_chunk(n)
                    for i in range(4 * n, 4 * n + 4):
                        proj_v(i)
                    av0 = attn_kloop(n, 0)
                    attn_normalize(n, 0, av0)
                    av1 = attn_kloop(n, 1)
                    attn_normalize(n, 1, av1)
                    wo_chunk(n)

    nc.compile()
    return nc


def _masks_np():
    m = np.zeros((4, 128, SC), dtype=np.float16)
    qq = np.arange(SC)[None, :]
    kk = np.arange(128)[:, None]
    for t in range(4):
        m[t] = ((128 * t + kk) <= qq).astype(np.float16)
    return m


def kernel(x, Wq, Wk, Wv, Wo):
    x = np.asarray(x, dtype=np.float32)
    Wq = np.asarray(Wq, dtype=np.float32)
    Wk = np.asarray(Wk, dtype=np.float32)
    Wv = np.asarray(Wv, dtype=np.float32)
    Wo = np.asarray(Wo, dtype=np.float32)

    if "nc" not in _CACHE:
        _CACHE["nc"] = _build()
    nc = _CACHE["nc"]

    masks = _masks_np()
    xT = [np.ascontiguousarray(x[b].T).astype(np.float16) for b in range(B)]
    in_maps = []
    for c in range(NCORES):
        b, g = c // 4, c % 4
        rows = slice(256 * g, 256 * g + 256)
        in_maps.append(
            {
                "xT": xT[b],
                "wqT": np.ascontiguousarray(Wq[rows].T).astype(np.float16),
                "wkT": np.ascontiguousarray(Wk[rows].T).astype(np.float16),
                "wvT": np.ascontiguousarray(Wv[rows].T).astype(np.float16),
                "woT": np.ascontiguousarray(Wo[:, rows].T).astype(np.float16),
                "mask": masks,
                "ones": np.ones((128, HPC), dtype=np.float16),
            }
        )

    trace = False
    if os.environ.get("KERNEL_TRACE") == "1":
        try:
            from trn_agent_boot.trn_boot import _ntff_profile_via_ctypes

            try:
                from antenv.axon_hooks import (
                    get_axon_ntff_profile_hook,
                    set_axon_ntff_profile_hook,
                )
            except ImportError:
                # this image's antenv lacks axon_hooks; provide the
                # 2-function registry bass_utils expects (test-only path)
                import types

                import antenv

                mod = types.ModuleType("antenv.axon_hooks")
                mod._hook = None

                def set_axon_ntff_profile_hook(h, _m=mod):
                    _m._hook = h

                def get_axon_ntff_profile_hook(_m=mod):
                    return _m._hook

                mod.set_axon_ntff_profile_hook = set_axon_ntff_profile_hook
                mod.get_axon_ntff_profile_hook = get_axon_ntff_profile_hook
                sys.modules["antenv.axon_hooks"] = mod
                antenv.axon_hooks = mod

            if get_axon_ntff_profile_hook() is None:
                set_axon_ntff_profile_hook(
                    _ntff_profile_via_ctypes("/opt/axon/libaxon_pjrt.so")
                )
            trace = True
        except Exception:
            trace = False

    res = run_bass_kernel_spmd(nc, in_maps, core_ids=list(range(NCORES)), trace=trace)
    _CACHE["exec_time_ns"] = res.exec_time_ns
    _CACHE["res"] = res
    y = np.zeros((B, S, D), dtype=np.float32)
    for c in range(NCORES):
        y[c // 4] += res.results[c]["y"].astype(np.float32)
    return y
